# revision 1
# baseline (speedup 1.0000x reference)
"""Trainium2 Bass kernel for nn_MixtureOfBidders.

Strategy: pure data-parallel over tokens (8 cores x 512 tokens), all weights
replicated per core. On device, everything runs in a transposed layout
[feature partitions, token free-dim]:

  - confidence head + top-2 auction + softmax routing computed on device (fp32)
  - base SwiGLU gate/up matmuls computed once per token (fp32r)
  - per-expert LoRA-gate contribution added in PSUM via an identity-matmul
    trick (PE accumulates base + lora in one PSUM bank, so the vector engine
    never does the add for the gate path)
  - h_wsum = sum_e we_e * silu(g_e) * u_e accumulated in bf16
  - shared base_down matmul factored out of the expert loop (done once on
    h_wsum); per-expert down-LoRA uses one-shot PSUM matmuls + DVE accumulate
"""

import functools
import sys

import numpy as np

sys.path.insert(0, "/opt/trn_rl_repo")

import ml_dtypes  # noqa: E402

import concourse.bass as bass  # noqa: E402
from concourse import bacc  # noqa: E402
import concourse.mybir as mybir  # noqa: E402
import concourse.tile as tile  # noqa: E402
from concourse.bass_utils import run_bass_kernel_spmd  # noqa: E402

B, S, H, I, E, TOPK, R = 4, 1024, 2048, 7168, 8, 2, 64
SCALING = 16.0 / 64.0
N_CORES = 8
N_TOK = B * S  # 4096
T = N_TOK // N_CORES  # 512 tokens per core
HC = H // 128  # 16 contraction chunks over H
IT = int(__import__('os').environ.get('KIT', I // 128))  # chunks over I
KSTAGE = int(__import__('os').environ.get('KSTAGE', 4))

F32 = mybir.dt.float32
F32R = mybir.dt.float32r
BF16 = mybir.dt.bfloat16
BFNP = ml_dtypes.bfloat16
AF = mybir.ActivationFunctionType
OP = mybir.AluOpType


def r32(ap):
    return ap.bitcast(F32R)


def build_module() -> bass.Bass:
    nc = bacc.Bacc("TRN2", target_bir_lowering=False)

    # ---- dram I/O (per core) ----
    xT = nc.dram_tensor("xT", [H, T], F32R, kind="ExternalInput")
    conf_wt = nc.dram_tensor("conf_wt", [H, E], F32, kind="ExternalInput")
    conf_b = nc.dram_tensor("conf_b", [E, 1], F32, kind="ExternalInput")
    wealth = nc.dram_tensor("wealth", [E, 1], F32, kind="ExternalInput")
    guA = nc.dram_tensor("guA", [E, H, 2 * R], F32R, kind="ExternalInput")
    guB = nc.dram_tensor("guB", [E, 2 * R, I], BF16, kind="ExternalInput")
    bgate = nc.dram_tensor("bgate", [H, I], F32R, kind="ExternalInput")
    bup = nc.dram_tensor("bup", [H, I], F32R, kind="ExternalInput")
    bdown = nc.dram_tensor("bdown", [I, H], BF16, kind="ExternalInput")
    dA = nc.dram_tensor("dA", [E, I, R], BF16, kind="ExternalInput")
    dB = nc.dram_tensor("dB", [E, R, H], BF16, kind="ExternalInput")
    ident = nc.dram_tensor("ident", [128, 128], BF16, kind="ExternalInput")
    outT = nc.dram_tensor("outT", [H, T], F32, kind="ExternalOutput")

    with tile.TileContext(nc) as tc:
        with (
            tc.tile_pool(name="consts", bufs=1) as consts,
            tc.tile_pool(name="dram", bufs=1, space="DRAM") as dpool,
            tc.tile_pool(name="pw", bufs=6, space="PSUM") as pw,
            tc.tile_pool(name="ptd", bufs=2, space="PSUM") as ptd,
            tc.tile_pool(name="acc", bufs=IT) as accp,
            tc.tile_pool(name="td", bufs=E) as tdp,
            tc.tile_pool(name="xp", bufs=1) as xp,
            tc.tile_pool(name="tA", bufs=E) as tAp,
            tc.tile_pool(name="web", bufs=E) as webp,
        ):
            id_sb = consts.tile([128, 128], BF16)
            nc.sync.dma_start(out=id_sb, in_=ident[:, :])
            cb_sb = consts.tile([E, 1], F32)
            nc.sync.dma_start(out=cb_sb, in_=conf_b[:, :])
            wl_sb = consts.tile([E, 1], F32)
            nc.sync.dma_start(out=wl_sb, in_=wealth[:, :])

            acc_t = [
                accp.tile([128, T], BF16, tag="acc", name=f"acc{i}")
                for i in range(IT)
            ]
            td_t = [
                tdp.tile([64, T], BF16, tag="td", name=f"td{i}")
                for i in range(E)
            ]

            # ---------- load x ----------
            x_sb = xp.tile([128, HC, T], F32R)
            nc.sync.dma_start(
                out=x_sb, in_=xT[:, :].rearrange("(c p) t -> p c t", p=128)
            )

            # ---------- routing (fp32, scoped pool) ----------
            we_b = []
            with tc.tile_pool(name="rt", bufs=2) as rt:
                cw_sb = rt.tile([128, HC, E], F32, tag="cw")
                nc.sync.dma_start(
                    out=cw_sb,
                    in_=conf_wt[:, :].rearrange("(c p) e -> p c e", p=128),
                )
                p_cf = pw.tile([128, T], F32, tag="big")
                for hc in range(HC):
                    nc.tensor.matmul(
                        p_cf[0:E, :],
                        cw_sb[:, hc, :],
                        x_sb[:, hc, :].bitcast(F32),
                        start=(hc == 0),
                        stop=(hc == HC - 1),
                    )
                conf = rt.tile([E, T], F32, tag="conf")
                nc.scalar.activation(conf, p_cf[0:E, :], AF.Sigmoid, bias=cb_sb)
                bids = rt.tile([E, T], F32, tag="bids")
                nc.vector.tensor_scalar(bids, conf, wl_sb, None, op0=OP.mult)

                def pfold_max(src, n, name, dtag):
                    """max over pairs of rows: [n, T] -> [n//2, T] via DRAM bounce
                    (DVE partition offsets must be 32-aligned, so realign in DRAM)."""
                    half = n // 2
                    scr = dpool.tile([n, T], F32, tag="folds", name=name + "s")
                    nc.sync.dma_start(out=scr, in_=src)
                    t = rt.tile([half, 2, T], F32, tag="foldt", name=name + "t")
                    s_ap = scr[:, :]
                    bap = bass.AP(
                        tensor=s_ap.tensor,
                        offset=s_ap.offset,
                        ap=[[T, half], [half * T, 2], [1, T]],
                    )
                    nc.sync.dma_start(out=t, in_=bap)
                    dst = rt.tile([half, T], F32, tag=dtag, name=name + "d")
                    nc.vector.tensor_tensor(dst, t[:, 0, :], t[:, 1, :], op=OP.max)
                    return dst

                def pmax8(src, tag):
                    a = pfold_max(src, 8, tag + "a", "foldd")
                    b = pfold_max(a, 4, tag + "b", "foldd")
                    return pfold_max(b, 2, tag + "c", tag + "res")

                def bcast_rows(row_ap, nrows, tag):
                    """broadcast [1, T] fp32 row -> [nrows, T] via DRAM bounce."""
                    scr = dpool.tile([1, T], F32, tag="scr", name="scr_" + tag)
                    nc.sync.dma_start(out=scr, in_=row_ap)
                    dst = rt.tile([nrows, T], F32, tag="bc", name="bc_" + tag)
                    src = scr[0:1, :]
                    bap = bass.AP(
                        tensor=src.tensor,
                        offset=src.offset,
                        ap=[[0, nrows]] + list(src.ap[1:]),
                    )
                    nc.sync.dma_start(out=dst, in_=bap)
                    return dst

                m1 = pmax8(bids, "m1")
                m1b = bcast_rows(m1, E, "m1b")
                mask1 = rt.tile([E, T], F32, tag="mask1")
                nc.vector.tensor_tensor(mask1, bids, m1b, op=OP.is_equal)
                bids2 = rt.tile([E, T], F32, tag="bids2")
                nc.vector.scalar_tensor_tensor(
                    bids2, mask1, -1e6, bids, op0=OP.mult, op1=OP.add
                )
                m2 = pmax8(bids2, "m2")
                m2b = bcast_rows(m2, E, "m2b")
                mask2 = rt.tile([E, T], F32, tag="mask2")
                nc.vector.tensor_tensor(mask2, bids2, m2b, op=OP.is_equal)

                d12 = rt.tile([1, T], F32, tag="d12")
                nc.vector.tensor_sub(d12, m1, m2)
                w1 = rt.tile([1, T], F32, tag="w1")
                nc.scalar.activation(w1, d12, AF.Sigmoid)
                w2 = rt.tile([1, T], F32, tag="w2")
                nc.scalar.activation(w2, d12, AF.Sigmoid, scale=-1.0)
                w1b = bcast_rows(w1, E, "w1b")
                w2b = bcast_rows(w2, E, "w2b")
                wea = rt.tile([E, T], F32, tag="wea")
                nc.vector.tensor_mul(wea, mask1, w1b)
                web8 = rt.tile([E, T], F32, tag="web8")
                nc.vector.tensor_mul(web8, mask2, w2b)
                we8 = rt.tile([E, T], BF16, tag="we8")
                nc.vector.tensor_add(we8, wea, web8)

                # broadcast each expert's weight row to 128 partitions (bf16)
                scr_we = dpool.tile([E, T], BF16, tag="scrwe")
                nc.sync.dma_start(out=scr_we, in_=we8)
                for e in range(E):
                    wt = webp.tile([128, T], BF16, tag="web", name=f"web{e}")
                    src = scr_we[e : e + 1, :]
                    bap = bass.AP(
                        tensor=src.tensor,
                        offset=src.offset,
                        ap=[[0, 128]] + list(src.ap[1:]),
                    )
                    nc.sync.dma_start(out=wt, in_=bap)
                    we_b.append(wt)

            # ---------- main loop (scoped pools) ----------
            with (
                tc.tile_pool(name="wga", bufs=2) as wga,
                tc.tile_pool(name="wgw", bufs=2) as wgw,
                tc.tile_pool(name="wb", bufs=2) as wbp,
                tc.tile_pool(name="wdA", bufs=2) as wdAp,
                tc.tile_pool(name="bsb", bufs=2) as bsb,
                tc.tile_pool(name="ew", bufs=4) as ew,
            ):
                # tA = x @ [gate_A | up_A]  -> [128=(Rg|Ru), T] per expert
                tAgu = []
                for e in range(E if KSTAGE >= 2 else 0):
                    ga_sb = wga.tile([128, HC, 2 * R], F32R, tag="guA")
                    nc.sync.dma_start(
                        out=ga_sb,
                        in_=guA[e, :, :].rearrange("(c p) r -> p c r", p=128),
                    )
                    p_tA = pw.tile([128, T], F32, tag="big")
                    for hc in range(HC):
                        nc.tensor.matmul(
                            p_tA,
                            ga_sb[:, hc, :],
                            x_sb[:, hc, :],
                            start=(hc == 0),
                            stop=(hc == HC - 1),
                        )
                    tAg_sb = tAp.tile([64, T], BF16, tag="tAg", name=f"tAg{e}")
                    nc.scalar.copy(tAg_sb, p_tA[0:64, :])
                    tAu_sb = tAp.tile([64, T], BF16, tag="tAu", name=f"tAu{e}")
                    nc.scalar.copy(tAu_sb, p_tA[64:128, :])
                    tAgu.append((tAg_sb, tAu_sb))

                for it in range(IT if KSTAGE >= 3 else 0):
                    bg_w = wgw.tile([128, HC, 128], F32R, tag="bgw")
                    nc.sync.dma_start(
                        out=bg_w,
                        in_=bgate[:, it * 128 : (it + 1) * 128].rearrange(
                            "(c p) i -> p c i", p=128
                        ),
                    )
                    bu_w = wgw.tile([128, HC, 128], F32R, tag="buw")
                    nc.sync.dma_start(
                        out=bu_w,
                        in_=bup[:, it * 128 : (it + 1) * 128].rearrange(
                            "(c p) i -> p c i", p=128
                        ),
                    )
                    p_bg = pw.tile([128, T], F32, tag="big")
                    p_bu = pw.tile([128, T], F32, tag="big")
                    for hc in range(HC):
                        nc.tensor.matmul(
                            p_bg,
                            bg_w[:, hc, :],
                            x_sb[:, hc, :],
                            start=(hc == 0),
                            stop=(hc == HC - 1),
                        )
                    for hc in range(HC):
                        nc.tensor.matmul(
                            p_bu,
                            bu_w[:, hc, :],
                            x_sb[:, hc, :],
                            start=(hc == 0),
                            stop=(hc == HC - 1),
                        )
                    bg_s = bsb.tile([128, T], BF16, tag="bgs")
                    nc.scalar.copy(bg_s, p_bg)
                    bu_s = bsb.tile([128, T], BF16, tag="bus")
                    nc.scalar.copy(bu_s, p_bu)

                    guB_s = wbp.tile([64, E, 2, 128], BF16, tag="guB")
                    nc.sync.dma_start(
                        out=guB_s,
                        in_=guB[:, :, it * 128 : (it + 1) * 128].rearrange(
                            "e (gu r) i -> r e gu i", gu=2
                        ),
                    )
                    dA_s = wdAp.tile([128, E, R], BF16, tag="dA")
                    nc.sync.dma_start(
                        out=dA_s,
                        in_=dA[:, it * 128 : (it + 1) * 128, :].rearrange(
                            "e p r -> p e r"
                        ),
                    )

                    p_td = None
                    for e in range(E):
                        # g_e = base_g + lora_g in PSUM (identity-matmul trick)
                        p_g = pw.tile([128, T], F32, tag="big")
                        nc.tensor.matmul(p_g, id_sb, bg_s, start=True, stop=False)
                        nc.tensor.matmul(
                            p_g,
                            guB_s[:, e, 0, :],
                            tAgu[e][0],
                            start=False,
                            stop=True,
                        )
                        # lora_u alone in PSUM
                        p_lu = pw.tile([128, T], F32, tag="big")
                        nc.tensor.matmul(
                            p_lu,
                            guB_s[:, e, 1, :],
                            tAgu[e][1],
                            start=True,
                            stop=True,
                        )
                        sg = ew.tile([128, T], BF16, tag="sg")
                        nc.scalar.activation(sg, p_g, AF.Silu)
                        u_t = ew.tile([128, T], BF16, tag="u")
                        nc.vector.scalar_tensor_tensor(
                            u_t, p_lu, 1.0, bu_s, op0=OP.bypass, op1=OP.add
                        )
                        h_t = ew.tile([128, T], BF16, tag="h")
                        nc.vector.tensor_mul(h_t, sg, u_t)
                        if e == 0:
                            hw_t = acc_t[it]
                            nc.vector.tensor_mul(hw_t, h_t, we_b[e])
                        else:
                            hw_t = ew.tile([128, T], BF16, tag="hw")
                            nc.vector.tensor_mul(hw_t, h_t, we_b[e])
                            nc.vector.tensor_add(acc_t[it], acc_t[it], hw_t)
                        # down-lora partial: td[e] += hw_e @ dA[e]
                        p_td = ptd.tile([64, T], F32, tag="ptd")
                        nc.tensor.matmul(
                            p_td, dA_s[:, e, :], hw_t, start=True, stop=True
                        )
                        if it == 0:
                            nc.vector.tensor_copy(td_t[e], p_td)
                        else:
                            nc.vector.tensor_add(td_t[e], td_t[e], p_td)

            # ---------- down projection ----------
            with (
                tc.tile_pool(name="wd", bufs=2) as wd,
                tc.tile_pool(name="wdB", bufs=2) as wdB,
                tc.tile_pool(name="osb", bufs=3) as osb,
            ):
                dB4 = dB[:, :, :].rearrange("e r h -> r e h")
                for hc in range(HC if KSTAGE >= 4 else 0):
                    bd_s = wd.tile([128, IT, 128], BF16, tag="bd")
                    nc.sync.dma_start(
                        out=bd_s,
                        in_=bdown[0 : IT * 128, hc * 128 : (hc + 1) * 128].rearrange(
                            "(c p) h -> p c h", p=128
                        ),
                    )
                    dB_s = wdB.tile([64, E, 128], BF16, tag="dB")
                    nc.sync.dma_start(
                        out=dB_s, in_=dB4[:, :, hc * 128 : (hc + 1) * 128]
                    )
                    p_o = pw.tile([128, T], F32, tag="big")
                    for it in range(IT):
                        nc.tensor.matmul(
                            p_o, bd_s[:, it, :], acc_t[it], start=(it == 0), stop=False
                        )
                    for e in range(E):
                        nc.tensor.matmul(
                            p_o, dB_s[:, e, :], td_t[e], start=False, stop=(e == E - 1)
                        )
                    o_s = osb.tile([128, T], F32, tag="o")
                    nc.scalar.copy(o_s, p_o)
                    nc.sync.dma_start(
                        out=outT[hc * 128 : (hc + 1) * 128, :], in_=o_s
                    )
    nc.compile()
    return nc


@functools.lru_cache(maxsize=1)
def _get_module():
    return build_module()


def _host_prep(inputs):
    f32 = np.float32
    x = np.ascontiguousarray(np.asarray(inputs["hidden_states"], f32)).reshape(
        N_TOK, H
    )
    gate_A = np.asarray(inputs["gate_A"], f32)
    gate_B = np.asarray(inputs["gate_B"], f32)
    up_A = np.asarray(inputs["up_A"], f32)
    up_B = np.asarray(inputs["up_B"], f32)
    down_A = np.asarray(inputs["down_A"], f32)
    down_B = np.asarray(inputs["down_B"], f32)

    shared = {
        "conf_wt": np.ascontiguousarray(np.asarray(inputs["conf_W"], f32).T),
        "conf_b": np.ascontiguousarray(
            np.asarray(inputs["conf_b"], f32).reshape(E, 1)
        ),
        "wealth": np.ascontiguousarray(
            np.asarray(inputs["expert_wealth"], f32).reshape(E, 1)
        ),
        "guA": np.ascontiguousarray(np.concatenate([gate_A, up_A], axis=2)),
        "guB": np.ascontiguousarray(
            (np.concatenate([gate_B, up_B], axis=1) * f32(SCALING)).astype(BFNP)
        ),
        "bgate": np.ascontiguousarray(np.asarray(inputs["base_gate"], f32)),
        "bup": np.ascontiguousarray(np.asarray(inputs["base_up"], f32)),
        "bdown": np.ascontiguousarray(
            np.asarray(inputs["base_down"], f32).astype(BFNP)
        ),
        "dA": np.ascontiguousarray(down_A.astype(BFNP)),
        "dB": np.ascontiguousarray((down_B * f32(SCALING)).astype(BFNP)),
        "ident": np.eye(128, dtype=BFNP),
    }
    in_maps = []
    for c in range(N_CORES):
        m = dict(shared)
        m["xT"] = np.ascontiguousarray(x[c * T : (c + 1) * T, :].T)
        in_maps.append(m)
    return in_maps


def kernel(**inputs) -> np.ndarray:
    nc = _get_module()
    in_maps = _host_prep(inputs)
    res = run_bass_kernel_spmd(nc, in_maps, core_ids=list(range(N_CORES)))
    parts = [np.asarray(r["outT"], np.float32).T for r in res.results]
    return np.concatenate(parts, axis=0).reshape(B, S, H)



# revision 10
# speedup vs baseline: 1.1217x; 1.1217x over previous
"""Trainium2 Bass kernel for nn_MixtureOfBidders (v2).

Data-parallel over tokens (8 cores x 512 tokens), weights replicated.
Device layout is transposed: [feature partitions, token free-dim].

v2 changes vs baseline:
  - all matmuls bf16 (fp32r ran as fp32-HIGH: no FWL, 224ns LDWEIGHTS tax)
  - conf logits keep fp32 accuracy via hi/lo bf16 split of x and conf_W
    (3 cross terms), so top-2 auction matches the fp32 reference
  - routing entirely on-chip: partition-spread matmuls + 32-aligned DVE
    max-folds + K=1/K=8 broadcast matmuls (no DRAM bounces)
  - identity-matmul trick on BOTH gate and up paths (PSUM carries
    base+lora), freeing the DVE of the u-path add
  - per-expert down-LoRA partials (td) accumulate directly in PSUM
    across all I-chunks (2 experts per bank via tile_position), the
    routing weight is applied once at the end (commutes with dA.T @ .)
  - td pairs feed pair-stacked dB matmuls in the down pass (K=128)
  - td matmuls software-pipelined 2 experts behind their h producers
"""

import functools
import os
import sys

import numpy as np

sys.path.insert(0, "/opt/trn_rl_repo")

import ml_dtypes  # noqa: E402

import concourse.bass as bass  # noqa: E402
from concourse import bacc  # noqa: E402
import concourse.mybir as mybir  # noqa: E402
import concourse.tile as tile  # noqa: E402
from concourse.bass_utils import run_bass_kernel_spmd  # noqa: E402

B, S, H, I, E, TOPK, R = 4, 1024, 2048, 7168, 8, 2, 64
SCALING = 16.0 / 64.0
N_CORES = 8
N_TOK = B * S  # 4096
T = N_TOK // N_CORES  # 512 tokens per core
HC = H // 128  # 16 contraction chunks over H
IT = I // 128  # 56 chunks over I
NP = E // 2  # 4 expert pairs

IDU = int(os.environ.get("IDU", "1"))  # 1: id-trick on up path
ACC_ENG = os.environ.get("ACC_ENG", "vector")  # vector|gpsimd for acc adds
TDLAG = int(os.environ.get("TDLAG", "2"))  # td matmul staggered this many experts

F32 = mybir.dt.float32
BF16 = mybir.dt.bfloat16
BFNP = ml_dtypes.bfloat16
AF = mybir.ActivationFunctionType
OP = mybir.AluOpType


def build_module() -> bass.Bass:
    nc = bacc.Bacc("TRN2", target_bir_lowering=False)

    # ---- dram I/O (per core) ----
    xh = nc.dram_tensor("xh", [H, T], BF16, kind="ExternalInput")
    xl = nc.dram_tensor("xl", [H, T], BF16, kind="ExternalInput")
    cwh = nc.dram_tensor("cwh", [H, E], BF16, kind="ExternalInput")
    cwl = nc.dram_tensor("cwl", [H, E], BF16, kind="ExternalInput")
    conf_b = nc.dram_tensor("conf_b", [E, 1], F32, kind="ExternalInput")
    wealth = nc.dram_tensor("wealth", [E, 1], F32, kind="ExternalInput")
    id8 = nc.dram_tensor("id8", [E, E], F32, kind="ExternalInput")
    seb = nc.dram_tensor("seb", [E, E, 128], BF16, kind="ExternalInput")
    spb = nc.dram_tensor("spb", [NP, E, 128], BF16, kind="ExternalInput")
    guA = nc.dram_tensor("guA", [E, H, 2 * R], BF16, kind="ExternalInput")
    guB = nc.dram_tensor("guB", [E, 2 * R, I], BF16, kind="ExternalInput")
    bgate = nc.dram_tensor("bgate", [H, I], BF16, kind="ExternalInput")
    bup = nc.dram_tensor("bup", [H, I], BF16, kind="ExternalInput")
    bdown = nc.dram_tensor("bdown", [I, H], BF16, kind="ExternalInput")
    dA = nc.dram_tensor("dA", [E, I, R], BF16, kind="ExternalInput")
    dBp = nc.dram_tensor("dBp", [NP, 2 * R, H], BF16, kind="ExternalInput")
    ident = nc.dram_tensor("ident", [128, 128], BF16, kind="ExternalInput")
    outT = nc.dram_tensor("outT", [H, T], F32, kind="ExternalOutput")

    with tile.TileContext(nc) as tc:
        with (
            tc.tile_pool(name="consts", bufs=1) as consts,
            tc.tile_pool(name="pw", bufs=4, space="PSUM") as pw,
            tc.tile_pool(name="tdp", bufs=NP, space="PSUM") as tdp,
            tc.tile_pool(name="xp", bufs=1) as xp,
            tc.tile_pool(name="tA", bufs=E) as tAp,
            tc.tile_pool(name="web", bufs=E + NP) as webp,
            tc.tile_pool(name="acc", bufs=IT) as accp,
        ):
            id_sb = consts.tile([128, 128], BF16)
            nc.sync.dma_start(out=id_sb, in_=ident[:, :])
            cb_sb = consts.tile([E, 1], F32)
            nc.sync.dma_start(out=cb_sb, in_=conf_b[:, :])
            wl_sb = consts.tile([E, 1], F32)
            nc.sync.dma_start(out=wl_sb, in_=wealth[:, :])
            id8_sb = consts.tile([E, E], F32)
            nc.sync.dma_start(out=id8_sb, in_=id8[:, :])
            seb_sb = consts.tile([E, E, 128], BF16)
            nc.sync.dma_start(out=seb_sb, in_=seb[:, :, :].rearrange("e k m -> k e m"))
            spb_sb = consts.tile([E, NP, 128], BF16)
            nc.sync.dma_start(out=spb_sb, in_=spb[:, :, :].rearrange("p k m -> k p m"))

            # persistent psum: 4 banks for td pairs
            td_t = [
                tdp.tile([128, T], F32, tag="td", name=f"td{p}") for p in range(NP)
            ]
            acc_t = [
                accp.tile([128, T], BF16, tag="acc", name=f"acc{i}")
                for i in range(IT)
            ]

            # ---------- load x (hi/lo bf16) ----------
            x_sb = xp.tile([128, HC, T], BF16)
            nc.sync.dma_start(
                out=x_sb, in_=xh[:, :].rearrange("(c p) t -> p c t", p=128)
            )

            we_b = []  # [128,T] bf16 per expert
            wep_b = []  # [128,T] bf16 per pair (rows 0:64 = e even, 64:128 = e odd)
            with tc.tile_pool(name="rt", bufs=1) as rt:
                xl_sb = rt.tile([128, HC, T], BF16, tag="xl")
                nc.sync.dma_start(
                    out=xl_sb, in_=xl[:, :].rearrange("(c p) t -> p c t", p=128)
                )
                cwh_sb = rt.tile([128, HC, E], BF16, tag="cwh")
                nc.sync.dma_start(
                    out=cwh_sb, in_=cwh[:, :].rearrange("(c p) e -> p c e", p=128)
                )
                cwl_sb = rt.tile([128, HC, E], BF16, tag="cwl")
                nc.sync.dma_start(
                    out=cwl_sb, in_=cwl[:, :].rearrange("(c p) e -> p c e", p=128)
                )

                # conf logits: xh@Wh + xl@Wh + xh@Wl  (fp32-accurate)
                p_cf = pw.tile([128, T], F32, tag="big", name="p_cf")
                terms = [(cwh_sb, x_sb), (cwh_sb, xl_sb), (cwl_sb, x_sb)]
                n = 0
                for w_sb, m_sb in terms:
                    for hc in range(HC):
                        nc.tensor.matmul(
                            p_cf[0:E, :],
                            w_sb[:, hc, :],
                            m_sb[:, hc, :],
                            start=(n == 0),
                            stop=(n == 3 * HC - 1),
                        )
                        n += 1
                conf = rt.tile([E, T], F32, tag="conf")
                nc.scalar.activation(conf, p_cf[0:E, :], AF.Sigmoid, bias=cb_sb)
                bids = rt.tile([E, T], F32, tag="bids")
                nc.vector.tensor_scalar(bids, conf, wl_sb, None, op0=OP.mult)

                # transpose bids to token space: [128 tok, 4 chunk, E] fp32
                TC = T // 128
                bidsT = rt.tile([128, TC, E], F32, tag="bidsT")
                for tc_i in range(TC):
                    p_bt = pw.tile([128, T], F32, tag="big", name=f"p_bt{tc_i}")
                    nc.tensor.transpose(
                        p_bt[:, 0:E], bids[:, tc_i * 128 : (tc_i + 1) * 128], id8_sb
                    )
                    nc.vector.tensor_copy(bidsT[:, tc_i, :], p_bt[:, 0:E])

                def fmax8(src, nametag):
                    """max over the expert free-dim: [128,TC,8] -> [128,TC,1]"""
                    m4 = rt.tile([128, TC, 4], F32, tag=nametag + "4", name=nametag + "4")
                    nc.vector.tensor_tensor(
                        m4, src[:, :, 0:4], src[:, :, 4:8], op=OP.max
                    )
                    m2_ = rt.tile([128, TC, 2], F32, tag=nametag + "2", name=nametag + "2")
                    nc.vector.tensor_tensor(
                        m2_, m4[:, :, 0:2], m4[:, :, 2:4], op=OP.max
                    )
                    m = rt.tile([128, TC, 1], F32, tag=nametag + "m", name=nametag + "m")
                    nc.vector.tensor_tensor(
                        m, m2_[:, :, 0:1], m2_[:, :, 1:2], op=OP.max
                    )
                    return m

                def bc(m):  # broadcast [128,TC,1] over expert free-dim
                    return m.broadcast_to([128, TC, E])

                m1 = fmax8(bidsT, "m1")
                mask1 = rt.tile([128, TC, E], F32, tag="mask1")
                nc.vector.tensor_tensor(mask1, bidsT, bc(m1), op=OP.is_equal)
                bids2 = rt.tile([128, TC, E], F32, tag="bids2")
                nc.vector.scalar_tensor_tensor(
                    bids2, mask1, -1e6, bidsT, op0=OP.mult, op1=OP.add
                )
                m2 = fmax8(bids2, "m2")
                mask2 = rt.tile([128, TC, E], F32, tag="mask2")
                nc.vector.tensor_tensor(mask2, bids2, bc(m2), op=OP.is_equal)

                d12 = rt.tile([128, TC, 1], F32, tag="d12")
                nc.vector.tensor_sub(d12, m1, m2)
                w1 = rt.tile([128, TC, 1], F32, tag="w1")
                nc.scalar.activation(w1, d12, AF.Sigmoid)
                w2 = rt.tile([128, TC, 1], F32, tag="w2")
                nc.scalar.activation(w2, d12, AF.Sigmoid, scale=-1.0)
                wea = rt.tile([128, TC, E], F32, tag="wea")
                nc.vector.tensor_mul(wea, mask1, bc(w1))
                web8 = rt.tile([128, TC, E], F32, tag="web8")
                nc.vector.tensor_mul(web8, mask2, bc(w2))
                we8T = rt.tile([128, TC, E], BF16, tag="we8T")
                nc.vector.tensor_add(we8T, wea, web8)

                # transpose back to [E, T] bf16
                we8 = rt.tile([E, T], BF16, tag="we8")
                for tc_i in range(TC):
                    p_wt = pw.tile([128, T], F32, tag="big", name=f"p_wt{tc_i}")
                    pv = p_wt.bitcast(BF16)[0:E, 0:128]
                    nc.tensor.transpose(pv, we8T[:, tc_i, :], id_sb)
                    nc.vector.tensor_copy(
                        we8[:, tc_i * 128 : (tc_i + 1) * 128], pv
                    )

                # broadcast each expert row to 128 partitions via K=8 select-matmul
                for e in range(E):
                    p_web = pw.tile([128, T], F32, tag="big", name=f"pweb{e}")
                    nc.tensor.matmul(p_web, seb_sb[:, e, :], we8, start=True, stop=True)
                    wt = webp.tile([128, T], BF16, tag="web", name=f"web{e}")
                    nc.scalar.copy(wt, p_web)
                    we_b.append(wt)
                for p in range(NP):
                    p_wep = pw.tile([128, T], F32, tag="big", name=f"pwep{p}")
                    nc.tensor.matmul(p_wep, spb_sb[:, p, :], we8, start=True, stop=True)
                    wt = webp.tile([128, T], BF16, tag="wep", name=f"wep{p}")
                    nc.scalar.copy(wt, p_wep)
                    wep_b.append(wt)

            # ---------- tA = x @ [gate_A | up_A] per expert ----------
            tAgu = []
            with tc.tile_pool(name="wga", bufs=2) as wga:
                for e in range(E):
                    ga_sb = wga.tile([128, HC, 2 * R], BF16, tag="guA")
                    nc.sync.dma_start(
                        out=ga_sb,
                        in_=guA[e, :, :].rearrange("(c p) r -> p c r", p=128),
                    )
                    p_tA = pw.tile([128, T], F32, tag="big", name=f"ptA{e}")
                    for hc in range(HC):
                        nc.tensor.matmul(
                            p_tA,
                            ga_sb[:, hc, :],
                            x_sb[:, hc, :],
                            start=(hc == 0),
                            stop=(hc == HC - 1),
                        )
                    tAg_sb = tAp.tile([64, T], BF16, tag="tAg", name=f"tAg{e}")
                    nc.scalar.copy(tAg_sb, p_tA[0:64, :])
                    tAu_sb = tAp.tile([64, T], BF16, tag="tAu", name=f"tAu{e}")
                    nc.scalar.copy(tAu_sb, p_tA[64:128, :])
                    tAgu.append((tAg_sb, tAu_sb))

            # ---------- main loop over I chunks ----------
            acc_fn = nc.gpsimd if ACC_ENG == "gpsimd" else nc.vector
            with (
                tc.tile_pool(name="wgw", bufs=2) as wgw,
                tc.tile_pool(name="wb", bufs=2) as wbp,
                tc.tile_pool(name="wdA", bufs=2) as wdAp,
                tc.tile_pool(name="bsb", bufs=2) as bsb,
                tc.tile_pool(name="ew", bufs=3) as ew,
            ):
                # td matmuls are emitted TDLAG experts behind their h
                # producers (crossing it boundaries) so the PE never waits
                # on the ACT->DVE chain that computes h.
                td_pending = []  # list of emit closures, FIFO

                def make_td(it_, e_, dA_tile, h_tile):
                    def go():
                        p = e_ // 2
                        half = (e_ % 2) * 64
                        nc.tensor.matmul(
                            td_t[p][half : half + 64, :],
                            dA_tile[:, e_, :],
                            h_tile,
                            start=(it_ == 0),
                            stop=(it_ == IT - 1),
                            tile_position=(0, half),
                            skip_group_check=True,
                        )

                    return go

                def pop_td():
                    if td_pending:
                        td_pending.pop(0)()

                for it in range(IT):
                    bg_w = wgw.tile([128, HC, 128], BF16, tag="bgw")
                    nc.sync.dma_start(
                        out=bg_w,
                        in_=bgate[:, it * 128 : (it + 1) * 128].rearrange(
                            "(c p) i -> p c i", p=128
                        ),
                    )
                    bu_w = wgw.tile([128, HC, 128], BF16, tag="buw")
                    nc.sync.dma_start(
                        out=bu_w,
                        in_=bup[:, it * 128 : (it + 1) * 128].rearrange(
                            "(c p) i -> p c i", p=128
                        ),
                    )
                    p_bg = pw.tile([128, T], F32, tag="big", name="p_bg")
                    p_bu = pw.tile([128, T], F32, tag="big", name="p_bu")
                    for hc in range(HC):
                        nc.tensor.matmul(
                            p_bg, bg_w[:, hc, :], x_sb[:, hc, :],
                            start=(hc == 0), stop=(hc == HC - 1),
                        )
                    pop_td()
                    for hc in range(HC):
                        nc.tensor.matmul(
                            p_bu, bu_w[:, hc, :], x_sb[:, hc, :],
                            start=(hc == 0), stop=(hc == HC - 1),
                        )
                    pop_td()
                    bg_s = bsb.tile([128, T], BF16, tag="bgs")
                    nc.scalar.copy(bg_s, p_bg)
                    bu_s = bsb.tile([128, T], BF16, tag="bus")
                    nc.scalar.copy(bu_s, p_bu)

                    guB_s = wbp.tile([64, E, 2, 128], BF16, tag="guB")
                    nc.sync.dma_start(
                        out=guB_s,
                        in_=guB[:, :, it * 128 : (it + 1) * 128].rearrange(
                            "e (gu r) i -> r e gu i", gu=2
                        ),
                    )
                    dA_s = wdAp.tile([128, E, R], BF16, tag="dA")
                    nc.sync.dma_start(
                        out=dA_s,
                        in_=dA[:, it * 128 : (it + 1) * 128, :].rearrange(
                            "e p r -> p e r"
                        ),
                    )

                    for e in range(E):
                        # g_e = base_g + lora_g in PSUM (identity-matmul trick)
                        p_g = pw.tile([128, T], F32, tag="big", name="p_g")
                        nc.tensor.matmul(p_g, id_sb, bg_s, start=True, stop=False)
                        nc.tensor.matmul(
                            p_g, guB_s[:, e, 0, :], tAgu[e][0],
                            start=False, stop=True,
                        )
                        p_u = pw.tile([128, T], F32, tag="big", name="p_u")
                        if IDU:
                            nc.tensor.matmul(p_u, id_sb, bu_s, start=True, stop=False)
                            nc.tensor.matmul(
                                p_u, guB_s[:, e, 1, :], tAgu[e][1],
                                start=False, stop=True,
                            )
                        else:
                            nc.tensor.matmul(
                                p_u, guB_s[:, e, 1, :], tAgu[e][1],
                                start=True, stop=True,
                            )
                        sg = ew.tile([128, T], BF16, tag="sg")
                        nc.scalar.activation(sg, p_g, AF.Silu)
                        if IDU:
                            h_t = ew.tile([128, T], BF16, tag="h")
                            nc.vector.tensor_mul(h_t, sg, p_u)
                        else:
                            u_t = ew.tile([128, T], BF16, tag="u")
                            nc.vector.scalar_tensor_tensor(
                                u_t, p_u, 1.0, bu_s, op0=OP.bypass, op1=OP.add
                            )
                            h_t = ew.tile([128, T], BF16, tag="h")
                            nc.vector.tensor_mul(h_t, sg, u_t)
                        td_pending.append(make_td(it, e, dA_s, h_t))
                        if len(td_pending) > TDLAG:
                            pop_td()
                        if e == 0:
                            nc.vector.tensor_mul(acc_t[it], h_t, we_b[e])
                        else:
                            hw_t = ew.tile([128, T], BF16, tag="hw")
                            nc.vector.tensor_mul(hw_t, h_t, we_b[e])
                            acc_fn.tensor_add(acc_t[it], acc_t[it], hw_t)
                while td_pending:
                    pop_td()

            # ---------- apply routing weights to td pairs ----------
            tdw = []
            with tc.tile_pool(name="tdw", bufs=NP) as tdwp:
                for p in range(NP):
                    t_w = tdwp.tile([128, T], BF16, tag="tdw", name=f"tdw{p}")
                    nc.vector.tensor_mul(t_w, td_t[p], wep_b[p])
                    tdw.append(t_w)

                # ---------- down projection ----------
                with (
                    tc.tile_pool(name="wd", bufs=2) as wd,
                    tc.tile_pool(name="wdB", bufs=2) as wdB,
                    tc.tile_pool(name="osb", bufs=3) as osb,
                ):
                    for hc in range(HC):
                        bd_s = wd.tile([128, IT, 128], BF16, tag="bd")
                        nc.sync.dma_start(
                            out=bd_s,
                            in_=bdown[
                                0 : IT * 128, hc * 128 : (hc + 1) * 128
                            ].rearrange("(c p) h -> p c h", p=128),
                        )
                        dB_s = wdB.tile([128, NP, 128], BF16, tag="dB")
                        nc.sync.dma_start(
                            out=dB_s,
                            in_=dBp[:, :, hc * 128 : (hc + 1) * 128].rearrange(
                                "p r h -> r p h"
                            ),
                        )
                        p_o = pw.tile([128, T], F32, tag="big", name="p_o")
                        for it in range(IT):
                            nc.tensor.matmul(
                                p_o, bd_s[:, it, :], acc_t[it],
                                start=(it == 0), stop=False,
                            )
                        for p in range(NP):
                            nc.tensor.matmul(
                                p_o, dB_s[:, p, :], tdw[p],
                                start=False, stop=(p == NP - 1),
                            )
                        o_s = osb.tile([128, T], F32, tag="o")
                        nc.scalar.copy(o_s, p_o)
                        nc.sync.dma_start(
                            out=outT[hc * 128 : (hc + 1) * 128, :], in_=o_s
                        )
    nc.compile()
    return nc


@functools.lru_cache(maxsize=1)
def _get_module():
    return build_module()


def _host_prep(inputs):
    f32 = np.float32
    x = np.ascontiguousarray(np.asarray(inputs["hidden_states"], f32)).reshape(
        N_TOK, H
    )
    x_hi = x.astype(BFNP)
    x_lo = (x - x_hi.astype(f32)).astype(BFNP)
    cw = np.asarray(inputs["conf_W"], f32).T  # [H, E]
    cw_hi = cw.astype(BFNP)
    cw_lo = (cw - cw_hi.astype(f32)).astype(BFNP)
    gate_A = np.asarray(inputs["gate_A"], f32)
    gate_B = np.asarray(inputs["gate_B"], f32)
    up_A = np.asarray(inputs["up_A"], f32)
    up_B = np.asarray(inputs["up_B"], f32)
    down_A = np.asarray(inputs["down_A"], f32)
    down_B = np.asarray(inputs["down_B"], f32) * f32(SCALING)

    seb = np.zeros((E, E, 128), BFNP)
    for e in range(E):
        seb[e, e, :] = 1.0
    spb = np.zeros((NP, E, 128), BFNP)
    for p in range(NP):
        spb[p, 2 * p, 0:64] = 1.0
        spb[p, 2 * p + 1, 64:128] = 1.0
    dBpair = np.stack(
        [np.concatenate([down_B[2 * p], down_B[2 * p + 1]], axis=0) for p in range(NP)]
    )  # [NP, 2R, H]

    shared = {
        "cwh": np.ascontiguousarray(cw_hi),
        "cwl": np.ascontiguousarray(cw_lo),
        "conf_b": np.ascontiguousarray(
            np.asarray(inputs["conf_b"], f32).reshape(E, 1)
        ),
        "wealth": np.ascontiguousarray(
            np.asarray(inputs["expert_wealth"], f32).reshape(E, 1)
        ),
        "id8": np.eye(E, dtype=f32),
        "seb": seb,
        "spb": spb,
        "guA": np.ascontiguousarray(
            np.concatenate([gate_A, up_A], axis=2).astype(BFNP)
        ),
        "guB": np.ascontiguousarray(
            (np.concatenate([gate_B, up_B], axis=1) * f32(SCALING)).astype(BFNP)
        ),
        "bgate": np.ascontiguousarray(np.asarray(inputs["base_gate"], f32).astype(BFNP)),
        "bup": np.ascontiguousarray(np.asarray(inputs["base_up"], f32).astype(BFNP)),
        "bdown": np.ascontiguousarray(
            np.asarray(inputs["base_down"], f32).astype(BFNP)
        ),
        "dA": np.ascontiguousarray(down_A.astype(BFNP)),
        "dBp": np.ascontiguousarray(dBpair.astype(BFNP)),
        "ident": np.eye(128, dtype=BFNP),
    }
    in_maps = []
    for c in range(N_CORES):
        m = dict(shared)
        m["xh"] = np.ascontiguousarray(x_hi[c * T : (c + 1) * T, :].T)
        m["xl"] = np.ascontiguousarray(x_lo[c * T : (c + 1) * T, :].T)
        in_maps.append(m)
    return in_maps


def kernel(**inputs) -> np.ndarray:
    nc = _get_module()
    in_maps = _host_prep(inputs)
    res = run_bass_kernel_spmd(nc, in_maps, core_ids=list(range(N_CORES)))
    parts = [np.asarray(r["outT"], np.float32).T for r in res.results]
    return np.concatenate(parts, axis=0).reshape(B, S, H)


# revision 20
# speedup vs baseline: 1.2477x; 1.1123x over previous
"""Trainium2 Bass kernel for nn_MixtureOfBidders (v2).

Data-parallel over tokens (8 cores x 512 tokens), weights replicated.
Device layout is transposed: [feature partitions, token free-dim].

v2 changes vs baseline:
  - all matmuls bf16 (fp32r ran as fp32-HIGH: no FWL, 224ns LDWEIGHTS tax)
  - conf logits keep fp32 accuracy via hi/lo bf16 split of x and conf_W
    (3 cross terms), so top-2 auction matches the fp32 reference
  - routing entirely on-chip: partition-spread matmuls + 32-aligned DVE
    max-folds + K=1/K=8 broadcast matmuls (no DRAM bounces)
  - identity-matmul trick on BOTH gate and up paths (PSUM carries
    base+lora), freeing the DVE of the u-path add
  - per-expert down-LoRA partials (td) accumulate directly in PSUM
    across all I-chunks (2 experts per bank via tile_position), the
    routing weight is applied once at the end (commutes with dA.T @ .)
  - td pairs feed pair-stacked dB matmuls in the down pass (K=128)
  - td matmuls software-pipelined 2 experts behind their h producers
"""

import functools
import os
import sys

import numpy as np

sys.path.insert(0, "/opt/trn_rl_repo")

import ml_dtypes  # noqa: E402

import concourse.bass as bass  # noqa: E402
from concourse import bacc  # noqa: E402
import concourse.mybir as mybir  # noqa: E402
import concourse.tile as tile  # noqa: E402
from concourse.bass_utils import run_bass_kernel_spmd  # noqa: E402

B, S, H, I, E, TOPK, R = 4, 1024, 2048, 7168, 8, 2, 64
SCALING = 16.0 / 64.0
N_CORES = 8
N_TOK = B * S  # 4096
T = N_TOK // N_CORES  # 512 tokens per core
HC = H // 128  # 16 contraction chunks over H
IT = I // 128  # 56 chunks over I
NP = E // 2  # 4 expert pairs

IDU = int(os.environ.get("IDU", "1"))  # 1: id-trick on up path
IDG = os.environ.get("IDG", "mm")  # mm: id-matmul | act: ACT copy into PSUM
ACC_ENG = os.environ.get("ACC_ENG", "vector")  # vector|gpsimd for acc adds
HW_ENG = os.environ.get("HW_ENG", "vector")  # vector|gpsimd for we mult
TDLAG = int(os.environ.get("TDLAG", "2"))  # td matmul staggered this many experts
WALAG = int(os.environ.get("WALAG", "0"))  # hw/acc DVE ops staggered this many
HLAG = max(TDLAG, WALAG) + 2

F32 = mybir.dt.float32
BF16 = mybir.dt.bfloat16
BFNP = ml_dtypes.bfloat16
AF = mybir.ActivationFunctionType
OP = mybir.AluOpType


def build_module() -> bass.Bass:
    nc = bacc.Bacc("TRN2", target_bir_lowering=False)

    # ---- dram I/O (per core) ----
    xh = nc.dram_tensor("xh", [H, T], BF16, kind="ExternalInput")
    xl = nc.dram_tensor("xl", [H, T], BF16, kind="ExternalInput")
    cwh = nc.dram_tensor("cwh", [H, E], BF16, kind="ExternalInput")
    cwl = nc.dram_tensor("cwl", [H, E], BF16, kind="ExternalInput")
    conf_b = nc.dram_tensor("conf_b", [E, 1], F32, kind="ExternalInput")
    wealth = nc.dram_tensor("wealth", [E, 1], F32, kind="ExternalInput")
    id8 = nc.dram_tensor("id8", [E, E], F32, kind="ExternalInput")
    seb = nc.dram_tensor("seb", [E, E, 128], BF16, kind="ExternalInput")
    spb = nc.dram_tensor("spb", [NP, E, 128], BF16, kind="ExternalInput")
    guA = nc.dram_tensor("guA", [E, H, 2 * R], BF16, kind="ExternalInput")
    guB = nc.dram_tensor("guB", [E, 2 * R, I], BF16, kind="ExternalInput")
    bgate = nc.dram_tensor("bgate", [H, I], BF16, kind="ExternalInput")
    bup = nc.dram_tensor("bup", [H, I], BF16, kind="ExternalInput")
    bdown = nc.dram_tensor("bdown", [I, H], BF16, kind="ExternalInput")
    dA = nc.dram_tensor("dA", [E, I, R], BF16, kind="ExternalInput")
    dBp = nc.dram_tensor("dBp", [NP, 2 * R, H], BF16, kind="ExternalInput")
    ident = nc.dram_tensor("ident", [128, 128], BF16, kind="ExternalInput")
    outT = nc.dram_tensor("outT", [H, T], F32, kind="ExternalOutput")

    with tile.TileContext(nc) as tc:
        with (
            tc.tile_pool(name="consts", bufs=1) as consts,
            tc.tile_pool(name="pw", bufs=4, space="PSUM") as pw,
            tc.tile_pool(name="tdp", bufs=NP, space="PSUM") as tdp,
            tc.tile_pool(name="xp", bufs=1) as xp,
            tc.tile_pool(name="tA", bufs=E) as tAp,
            tc.tile_pool(name="web", bufs=E + NP) as webp,
            tc.tile_pool(name="acc", bufs=IT) as accp,
        ):
            id_sb = consts.tile([128, 128], BF16)
            nc.sync.dma_start(out=id_sb, in_=ident[:, :])
            cb_sb = consts.tile([E, 1], F32)
            nc.sync.dma_start(out=cb_sb, in_=conf_b[:, :])
            wl_sb = consts.tile([E, 1], F32)
            nc.sync.dma_start(out=wl_sb, in_=wealth[:, :])
            id8_sb = consts.tile([E, E], F32)
            nc.sync.dma_start(out=id8_sb, in_=id8[:, :])
            seb_sb = consts.tile([E, E, 128], BF16)
            nc.sync.dma_start(out=seb_sb, in_=seb[:, :, :].rearrange("e k m -> k e m"))
            spb_sb = consts.tile([E, NP, 128], BF16)
            nc.sync.dma_start(out=spb_sb, in_=spb[:, :, :].rearrange("p k m -> k p m"))

            # persistent psum: 4 banks for td pairs
            td_t = [
                tdp.tile([128, T], F32, tag="td", name=f"td{p}") for p in range(NP)
            ]
            acc_t = [
                accp.tile([128, T], BF16, tag="acc", name=f"acc{i}")
                for i in range(IT)
            ]

            # ---------- load x (hi/lo bf16) ----------
            x_sb = xp.tile([128, HC, T], BF16)
            nc.sync.dma_start(
                out=x_sb, in_=xh[:, :].rearrange("(c p) t -> p c t", p=128)
            )

            we_b = []  # [128,T] bf16 per expert
            wep_b = []  # [128,T] bf16 per pair (rows 0:64 = e even, 64:128 = e odd)
            with tc.tile_pool(name="rt", bufs=1) as rt:
                xl_sb = rt.tile([128, HC, T], BF16, tag="xl")
                nc.sync.dma_start(
                    out=xl_sb, in_=xl[:, :].rearrange("(c p) t -> p c t", p=128)
                )
                cwh_sb = rt.tile([128, HC, E], BF16, tag="cwh")
                nc.sync.dma_start(
                    out=cwh_sb, in_=cwh[:, :].rearrange("(c p) e -> p c e", p=128)
                )
                cwl_sb = rt.tile([128, HC, E], BF16, tag="cwl")
                nc.sync.dma_start(
                    out=cwl_sb, in_=cwl[:, :].rearrange("(c p) e -> p c e", p=128)
                )

                # conf logits: xh@Wh + xl@Wh + xh@Wl  (fp32-accurate)
                p_cf = pw.tile([128, T], F32, tag="big", name="p_cf")
                terms = [(cwh_sb, x_sb), (cwh_sb, xl_sb), (cwl_sb, x_sb)]
                n = 0
                for w_sb, m_sb in terms:
                    for hc in range(HC):
                        nc.tensor.matmul(
                            p_cf[0:E, :],
                            w_sb[:, hc, :],
                            m_sb[:, hc, :],
                            start=(n == 0),
                            stop=(n == 3 * HC - 1),
                        )
                        n += 1
                conf = rt.tile([E, T], F32, tag="conf")
                nc.scalar.activation(conf, p_cf[0:E, :], AF.Sigmoid, bias=cb_sb)
                bids = rt.tile([E, T], F32, tag="bids")
                nc.vector.tensor_scalar(bids, conf, wl_sb, None, op0=OP.mult)

                # transpose bids to token space: [128 tok, 4 chunk, E] fp32
                TC = T // 128
                bidsT = rt.tile([128, TC, E], F32, tag="bidsT")
                for tc_i in range(TC):
                    p_bt = pw.tile([128, T], F32, tag="big", name=f"p_bt{tc_i}")
                    nc.tensor.transpose(
                        p_bt[:, 0:E], bids[:, tc_i * 128 : (tc_i + 1) * 128], id8_sb
                    )
                    nc.vector.tensor_copy(bidsT[:, tc_i, :], p_bt[:, 0:E])

                def fmax8(src, nametag):
                    """max over the expert free-dim: [128,TC,8] -> [128,TC,1]"""
                    m4 = rt.tile([128, TC, 4], F32, tag=nametag + "4", name=nametag + "4")
                    nc.vector.tensor_tensor(
                        m4, src[:, :, 0:4], src[:, :, 4:8], op=OP.max
                    )
                    m2_ = rt.tile([128, TC, 2], F32, tag=nametag + "2", name=nametag + "2")
                    nc.vector.tensor_tensor(
                        m2_, m4[:, :, 0:2], m4[:, :, 2:4], op=OP.max
                    )
                    m = rt.tile([128, TC, 1], F32, tag=nametag + "m", name=nametag + "m")
                    nc.vector.tensor_tensor(
                        m, m2_[:, :, 0:1], m2_[:, :, 1:2], op=OP.max
                    )
                    return m

                def bc(m):  # broadcast [128,TC,1] over expert free-dim
                    return m.broadcast_to([128, TC, E])

                m1 = fmax8(bidsT, "m1")
                mask1 = rt.tile([128, TC, E], F32, tag="mask1")
                nc.vector.tensor_tensor(mask1, bidsT, bc(m1), op=OP.is_equal)
                bids2 = rt.tile([128, TC, E], F32, tag="bids2")
                nc.vector.scalar_tensor_tensor(
                    bids2, mask1, -1e6, bidsT, op0=OP.mult, op1=OP.add
                )
                m2 = fmax8(bids2, "m2")
                mask2 = rt.tile([128, TC, E], F32, tag="mask2")
                nc.vector.tensor_tensor(mask2, bids2, bc(m2), op=OP.is_equal)

                d12 = rt.tile([128, TC, 1], F32, tag="d12")
                nc.vector.tensor_sub(d12, m1, m2)
                w1 = rt.tile([128, TC, 1], F32, tag="w1")
                nc.scalar.activation(w1, d12, AF.Sigmoid)
                w2 = rt.tile([128, TC, 1], F32, tag="w2")
                nc.scalar.activation(w2, d12, AF.Sigmoid, scale=-1.0)
                wea = rt.tile([128, TC, E], F32, tag="wea")
                nc.vector.tensor_mul(wea, mask1, bc(w1))
                web8 = rt.tile([128, TC, E], F32, tag="web8")
                nc.vector.tensor_mul(web8, mask2, bc(w2))
                we8T = rt.tile([128, TC, E], BF16, tag="we8T")
                nc.vector.tensor_add(we8T, wea, web8)

                # transpose back to [E, T] bf16
                we8 = rt.tile([E, T], BF16, tag="we8")
                for tc_i in range(TC):
                    p_wt = pw.tile([128, T], F32, tag="big", name=f"p_wt{tc_i}")
                    pv = p_wt.bitcast(BF16)[0:E, 0:128]
                    nc.tensor.transpose(pv, we8T[:, tc_i, :], id_sb)
                    nc.vector.tensor_copy(
                        we8[:, tc_i * 128 : (tc_i + 1) * 128], pv
                    )

                # broadcast each expert row to 128 partitions via K=8 select-matmul
                for e in range(E):
                    p_web = pw.tile([128, T], F32, tag="big", name=f"pweb{e}")
                    nc.tensor.matmul(p_web, seb_sb[:, e, :], we8, start=True, stop=True)
                    wt = webp.tile([128, T], BF16, tag="web", name=f"web{e}")
                    nc.scalar.copy(wt, p_web)
                    we_b.append(wt)
                for p in range(NP):
                    p_wep = pw.tile([128, T], F32, tag="big", name=f"pwep{p}")
                    nc.tensor.matmul(p_wep, spb_sb[:, p, :], we8, start=True, stop=True)
                    wt = webp.tile([128, T], BF16, tag="wep", name=f"wep{p}")
                    nc.scalar.copy(wt, p_wep)
                    wep_b.append(wt)

            # ---------- tA = x @ [gate_A | up_A] per expert ----------
            tAgu = []
            with tc.tile_pool(name="wga", bufs=2) as wga:
                for e in range(E):
                    ga_sb = wga.tile([128, HC, 2 * R], BF16, tag="guA")
                    nc.sync.dma_start(
                        out=ga_sb,
                        in_=guA[e, :, :].rearrange("(c p) r -> p c r", p=128),
                    )
                    p_tA = pw.tile([128, T], F32, tag="big", name=f"ptA{e}")
                    for hc in range(HC):
                        nc.tensor.matmul(
                            p_tA,
                            ga_sb[:, hc, :],
                            x_sb[:, hc, :],
                            start=(hc == 0),
                            stop=(hc == HC - 1),
                        )
                    tAg_sb = tAp.tile([64, T], BF16, tag="tAg", name=f"tAg{e}")
                    nc.scalar.copy(tAg_sb, p_tA[0:64, :])
                    tAu_sb = tAp.tile([64, T], BF16, tag="tAu", name=f"tAu{e}")
                    nc.scalar.copy(tAu_sb, p_tA[64:128, :])
                    tAgu.append((tAg_sb, tAu_sb))

            # ---------- main loop over I chunks ----------
            acc_fn = nc.gpsimd if ACC_ENG == "gpsimd" else nc.vector
            hw_fn = nc.gpsimd if HW_ENG == "gpsimd" else nc.vector
            with (
                tc.tile_pool(name="wgw", bufs=2) as wgw,
                tc.tile_pool(name="wb", bufs=2) as wbp,
                tc.tile_pool(name="wdA", bufs=2) as wdAp,
                tc.tile_pool(name="bsb", bufs=2) as bsb,
                tc.tile_pool(name="ew", bufs=3) as ew,
            ):
                # td matmuls are emitted TDLAG experts behind their h
                # producers (crossing it boundaries) so the PE never waits
                # on the ACT->DVE chain that computes h. The hw/acc DVE ops
                # are likewise deferred WALAG experts so queued h ops (which
                # gate td matmuls) aren't stuck behind them on the in-order
                # DVE.
                td_pending = []  # list of emit closures, FIFO
                wa_pending = []  # deferred hw/acc emits, FIFO

                def make_td(it_, e_, dA_tile, h_tile):
                    def go():
                        p = e_ // 2
                        half = (e_ % 2) * 64
                        nc.tensor.matmul(
                            td_t[p][half : half + 64, :],
                            dA_tile[:, e_, :],
                            h_tile,
                            start=(it_ == 0),
                            stop=(it_ == IT - 1),
                            tile_position=(0, half),
                            skip_group_check=True,
                        )

                    return go

                def pop_td():
                    if td_pending:
                        td_pending.pop(0)()

                def make_wa(it_, e_, h_tile):
                    def go():
                        if e_ == 0:
                            hw_fn.tensor_mul(acc_t[it_], h_tile, we_b[e_])
                        else:
                            hw_t = ew.tile(
                                [128, T], BF16, tag="hw", bufs=WALAG + 2,
                                name="hw_t",
                            )
                            hw_fn.tensor_mul(hw_t, h_tile, we_b[e_])
                            acc_fn.tensor_add(acc_t[it_], acc_t[it_], hw_t)

                    return go

                def pop_wa():
                    if wa_pending:
                        wa_pending.pop(0)()

                for it in range(IT):
                    bg_w = wgw.tile([128, HC, 128], BF16, tag="bgw")
                    nc.sync.dma_start(
                        out=bg_w,
                        in_=bgate[:, it * 128 : (it + 1) * 128].rearrange(
                            "(c p) i -> p c i", p=128
                        ),
                    )
                    bu_w = wgw.tile([128, HC, 128], BF16, tag="buw")
                    nc.sync.dma_start(
                        out=bu_w,
                        in_=bup[:, it * 128 : (it + 1) * 128].rearrange(
                            "(c p) i -> p c i", p=128
                        ),
                    )
                    p_bg = pw.tile([128, T], F32, tag="big", name="p_bg")
                    p_bu = pw.tile([128, T], F32, tag="big", name="p_bu")
                    for hc in range(HC):
                        nc.tensor.matmul(
                            p_bg, bg_w[:, hc, :], x_sb[:, hc, :],
                            start=(hc == 0), stop=(hc == HC - 1),
                        )
                    pop_td()
                    for hc in range(HC):
                        nc.tensor.matmul(
                            p_bu, bu_w[:, hc, :], x_sb[:, hc, :],
                            start=(hc == 0), stop=(hc == HC - 1),
                        )
                    pop_td()
                    bg_s = bsb.tile([128, T], BF16, tag="bgs")
                    nc.scalar.copy(bg_s, p_bg)
                    bu_s = bsb.tile([128, T], BF16, tag="bus")
                    nc.scalar.copy(bu_s, p_bu)

                    guB_s = wbp.tile([64, E, 2, 128], BF16, tag="guB")
                    nc.sync.dma_start(
                        out=guB_s,
                        in_=guB[:, :, it * 128 : (it + 1) * 128].rearrange(
                            "e (gu r) i -> r e gu i", gu=2
                        ),
                    )
                    dA_s = wdAp.tile([128, E, R], BF16, tag="dA")
                    nc.sync.dma_start(
                        out=dA_s,
                        in_=dA[:, it * 128 : (it + 1) * 128, :].rearrange(
                            "e p r -> p e r"
                        ),
                    )

                    for e in range(E):
                        # g_e = base_g + lora_g in PSUM (identity-matmul trick,
                        # or ACT writes the base into the bank and the lora
                        # matmul accumulates on top)
                        p_g = pw.tile([128, T], F32, tag="big", name="p_g")
                        if IDG == "act":
                            nc.scalar.copy(p_g, bg_s)
                            nc.tensor.matmul(
                                p_g, guB_s[:, e, 0, :], tAgu[e][0],
                                start=False, stop=True, skip_group_check=True,
                            )
                        else:
                            nc.tensor.matmul(p_g, id_sb, bg_s, start=True, stop=False)
                            nc.tensor.matmul(
                                p_g, guB_s[:, e, 0, :], tAgu[e][0],
                                start=False, stop=True,
                            )
                        p_u = pw.tile([128, T], F32, tag="big", name="p_u")
                        if IDU:
                            nc.tensor.matmul(p_u, id_sb, bu_s, start=True, stop=False)
                            nc.tensor.matmul(
                                p_u, guB_s[:, e, 1, :], tAgu[e][1],
                                start=False, stop=True,
                            )
                        else:
                            nc.tensor.matmul(
                                p_u, guB_s[:, e, 1, :], tAgu[e][1],
                                start=True, stop=True,
                            )
                        sg = ew.tile([128, T], BF16, tag="sg")
                        nc.scalar.activation(sg, p_g, AF.Silu)
                        if IDU:
                            h_t = ew.tile([128, T], BF16, tag="h", bufs=HLAG)
                            nc.vector.tensor_mul(h_t, sg, p_u)
                        else:
                            u_t = ew.tile([128, T], BF16, tag="u")
                            nc.vector.scalar_tensor_tensor(
                                u_t, p_u, 1.0, bu_s, op0=OP.bypass, op1=OP.add
                            )
                            h_t = ew.tile([128, T], BF16, tag="h", bufs=HLAG)
                            nc.vector.tensor_mul(h_t, sg, u_t)
                        td_pending.append(make_td(it, e, dA_s, h_t))
                        if len(td_pending) > TDLAG:
                            pop_td()
                        wa_pending.append(make_wa(it, e, h_t))
                        if len(wa_pending) > WALAG:
                            pop_wa()
                while td_pending:
                    pop_td()
                while wa_pending:
                    pop_wa()

            # ---------- apply routing weights to td pairs ----------
            tdw = []
            with tc.tile_pool(name="tdw", bufs=NP) as tdwp:
                for p in range(NP):
                    t_w = tdwp.tile([128, T], BF16, tag="tdw", name=f"tdw{p}")
                    nc.vector.tensor_mul(t_w, td_t[p], wep_b[p])
                    tdw.append(t_w)

                # ---------- down projection ----------
                with (
                    tc.tile_pool(name="wd", bufs=2) as wd,
                    tc.tile_pool(name="wdB", bufs=2) as wdB,
                    tc.tile_pool(name="osb", bufs=3) as osb,
                ):
                    for hc in range(HC):
                        bd_s = wd.tile([128, IT, 128], BF16, tag="bd")
                        nc.sync.dma_start(
                            out=bd_s,
                            in_=bdown[
                                0 : IT * 128, hc * 128 : (hc + 1) * 128
                            ].rearrange("(c p) h -> p c h", p=128),
                        )
                        dB_s = wdB.tile([128, NP, 128], BF16, tag="dB")
                        nc.sync.dma_start(
                            out=dB_s,
                            in_=dBp[:, :, hc * 128 : (hc + 1) * 128].rearrange(
                                "p r h -> r p h"
                            ),
                        )
                        p_o = pw.tile([128, T], F32, tag="big", name="p_o")
                        for it in range(IT):
                            nc.tensor.matmul(
                                p_o, bd_s[:, it, :], acc_t[it],
                                start=(it == 0), stop=False,
                            )
                        for p in range(NP):
                            nc.tensor.matmul(
                                p_o, dB_s[:, p, :], tdw[p],
                                start=False, stop=(p == NP - 1),
                            )
                        o_s = osb.tile([128, T], F32, tag="o")
                        nc.scalar.copy(o_s, p_o)
                        nc.sync.dma_start(
                            out=outT[hc * 128 : (hc + 1) * 128, :], in_=o_s
                        )
    nc.compile()
    return nc


@functools.lru_cache(maxsize=1)
def _get_module():
    return build_module()


def _host_prep(inputs):
    f32 = np.float32
    x = np.ascontiguousarray(np.asarray(inputs["hidden_states"], f32)).reshape(
        N_TOK, H
    )
    x_hi = x.astype(BFNP)
    x_lo = (x - x_hi.astype(f32)).astype(BFNP)
    cw = np.asarray(inputs["conf_W"], f32).T  # [H, E]
    cw_hi = cw.astype(BFNP)
    cw_lo = (cw - cw_hi.astype(f32)).astype(BFNP)
    gate_A = np.asarray(inputs["gate_A"], f32)
    gate_B = np.asarray(inputs["gate_B"], f32)
    up_A = np.asarray(inputs["up_A"], f32)
    up_B = np.asarray(inputs["up_B"], f32)
    down_A = np.asarray(inputs["down_A"], f32)
    down_B = np.asarray(inputs["down_B"], f32) * f32(SCALING)

    seb = np.zeros((E, E, 128), BFNP)
    for e in range(E):
        seb[e, e, :] = 1.0
    spb = np.zeros((NP, E, 128), BFNP)
    for p in range(NP):
        spb[p, 2 * p, 0:64] = 1.0
        spb[p, 2 * p + 1, 64:128] = 1.0
    dBpair = np.stack(
        [np.concatenate([down_B[2 * p], down_B[2 * p + 1]], axis=0) for p in range(NP)]
    )  # [NP, 2R, H]

    shared = {
        "cwh": np.ascontiguousarray(cw_hi),
        "cwl": np.ascontiguousarray(cw_lo),
        "conf_b": np.ascontiguousarray(
            np.asarray(inputs["conf_b"], f32).reshape(E, 1)
        ),
        "wealth": np.ascontiguousarray(
            np.asarray(inputs["expert_wealth"], f32).reshape(E, 1)
        ),
        "id8": np.eye(E, dtype=f32),
        "seb": seb,
        "spb": spb,
        "guA": np.ascontiguousarray(
            np.concatenate([gate_A, up_A], axis=2).astype(BFNP)
        ),
        "guB": np.ascontiguousarray(
            (np.concatenate([gate_B, up_B], axis=1) * f32(SCALING)).astype(BFNP)
        ),
        "bgate": np.ascontiguousarray(np.asarray(inputs["base_gate"], f32).astype(BFNP)),
        "bup": np.ascontiguousarray(np.asarray(inputs["base_up"], f32).astype(BFNP)),
        "bdown": np.ascontiguousarray(
            np.asarray(inputs["base_down"], f32).astype(BFNP)
        ),
        "dA": np.ascontiguousarray(down_A.astype(BFNP)),
        "dBp": np.ascontiguousarray(dBpair.astype(BFNP)),
        "ident": np.eye(128, dtype=BFNP),
    }
    in_maps = []
    for c in range(N_CORES):
        m = dict(shared)
        m["xh"] = np.ascontiguousarray(x_hi[c * T : (c + 1) * T, :].T)
        m["xl"] = np.ascontiguousarray(x_lo[c * T : (c + 1) * T, :].T)
        in_maps.append(m)
    return in_maps


def kernel(**inputs) -> np.ndarray:
    nc = _get_module()
    in_maps = _host_prep(inputs)
    res = run_bass_kernel_spmd(nc, in_maps, core_ids=list(range(N_CORES)))
    parts = [np.asarray(r["outT"], np.float32).T for r in res.results]
    return np.concatenate(parts, axis=0).reshape(B, S, H)


# revision 24
# speedup vs baseline: 1.2591x; 1.0091x over previous
"""Trainium2 Bass kernel for nn_MixtureOfBidders (v2).

Data-parallel over tokens (8 cores x 512 tokens), weights replicated.
Device layout is transposed: [feature partitions, token free-dim].

v2 changes vs baseline:
  - all matmuls bf16 (fp32r ran as fp32-HIGH: no FWL, 224ns LDWEIGHTS tax)
  - conf logits keep fp32 accuracy via hi/lo bf16 split of x and conf_W
    (3 cross terms), so top-2 auction matches the fp32 reference
  - routing entirely on-chip: partition-spread matmuls + 32-aligned DVE
    max-folds + K=1/K=8 broadcast matmuls (no DRAM bounces)
  - identity-matmul trick on BOTH gate and up paths (PSUM carries
    base+lora), freeing the DVE of the u-path add
  - per-expert down-LoRA partials (td) accumulate directly in PSUM
    across all I-chunks (2 experts per bank via tile_position), the
    routing weight is applied once at the end (commutes with dA.T @ .)
  - td pairs feed pair-stacked dB matmuls in the down pass (K=128)
  - td matmuls software-pipelined 2 experts behind their h producers
"""

import functools
import os
import sys

import numpy as np

sys.path.insert(0, "/opt/trn_rl_repo")

import ml_dtypes  # noqa: E402

import concourse.bass as bass  # noqa: E402
from concourse import bacc  # noqa: E402
import concourse.mybir as mybir  # noqa: E402
import concourse.tile as tile  # noqa: E402
from concourse.bass_utils import run_bass_kernel_spmd  # noqa: E402

B, S, H, I, E, TOPK, R = 4, 1024, 2048, 7168, 8, 2, 64
SCALING = 16.0 / 64.0
N_CORES = 8
N_TOK = B * S  # 4096
T = N_TOK // N_CORES  # 512 tokens per core
HC = H // 128  # 16 contraction chunks over H
IT = I // 128  # 56 chunks over I
NP = E // 2  # 4 expert pairs

IDU = int(os.environ.get("IDU", "1"))  # 1: id-trick on up path
IDG = os.environ.get("IDG", "mm")  # mm: id-matmul | act: ACT copy into PSUM
ACC_ENG = os.environ.get("ACC_ENG", "vector")  # vector|gpsimd for acc adds
HW_ENG = os.environ.get("HW_ENG", "vector")  # vector|gpsimd for we mult
TDLAG = int(os.environ.get("TDLAG", "2"))  # td matmul staggered this many experts
WALAG = int(os.environ.get("WALAG", "0"))  # hw/acc DVE ops staggered this many
HLAG = max(TDLAG, WALAG) + 2

F32 = mybir.dt.float32
BF16 = mybir.dt.bfloat16
BFNP = ml_dtypes.bfloat16
AF = mybir.ActivationFunctionType
OP = mybir.AluOpType


def build_module() -> bass.Bass:
    nc = bacc.Bacc("TRN2", target_bir_lowering=False)

    # ---- dram I/O (per core) ----
    xh = nc.dram_tensor("xh", [H, T], BF16, kind="ExternalInput")
    xl = nc.dram_tensor("xl", [H, T], BF16, kind="ExternalInput")
    cwh = nc.dram_tensor("cwh", [H, E], BF16, kind="ExternalInput")
    cwl = nc.dram_tensor("cwl", [H, E], BF16, kind="ExternalInput")
    conf_b = nc.dram_tensor("conf_b", [E, 1], F32, kind="ExternalInput")
    wealth = nc.dram_tensor("wealth", [E, 1], F32, kind="ExternalInput")
    id8 = nc.dram_tensor("id8", [E, E], F32, kind="ExternalInput")
    seb = nc.dram_tensor("seb", [E, E, 128], BF16, kind="ExternalInput")
    spb = nc.dram_tensor("spb", [NP, E, 128], BF16, kind="ExternalInput")
    guA = nc.dram_tensor("guA", [E, H, 2 * R], BF16, kind="ExternalInput")
    guB = nc.dram_tensor("guB", [E, 2 * R, I], BF16, kind="ExternalInput")
    bgate = nc.dram_tensor("bgate", [H, I], BF16, kind="ExternalInput")
    bup = nc.dram_tensor("bup", [H, I], BF16, kind="ExternalInput")
    bdown = nc.dram_tensor("bdown", [I, H], BF16, kind="ExternalInput")
    dA = nc.dram_tensor("dA", [E, I, R], BF16, kind="ExternalInput")
    dBp = nc.dram_tensor("dBp", [NP, 2 * R, H], BF16, kind="ExternalInput")
    ident = nc.dram_tensor("ident", [128, 128], BF16, kind="ExternalInput")
    outT = nc.dram_tensor("outT", [H, T], F32, kind="ExternalOutput")

    with tile.TileContext(nc) as tc:
        with (
            tc.tile_pool(name="consts", bufs=1) as consts,
            tc.tile_pool(name="pw", bufs=4, space="PSUM") as pw,
            tc.tile_pool(name="tdp", bufs=NP, space="PSUM") as tdp,
            tc.tile_pool(name="xp", bufs=1) as xp,
            tc.tile_pool(name="tA", bufs=E) as tAp,
            tc.tile_pool(name="web", bufs=E + NP) as webp,
            tc.tile_pool(name="acc", bufs=IT) as accp,
        ):
            id_sb = consts.tile([128, 128], BF16)
            nc.sync.dma_start(out=id_sb, in_=ident[:, :])
            cb_sb = consts.tile([E, 1], F32)
            nc.sync.dma_start(out=cb_sb, in_=conf_b[:, :])
            wl_sb = consts.tile([E, 1], F32)
            nc.sync.dma_start(out=wl_sb, in_=wealth[:, :])
            id8_sb = consts.tile([E, E], F32)
            nc.sync.dma_start(out=id8_sb, in_=id8[:, :])
            seb_sb = consts.tile([E, E, 128], BF16)
            nc.sync.dma_start(out=seb_sb, in_=seb[:, :, :].rearrange("e k m -> k e m"))
            spb_sb = consts.tile([E, NP, 128], BF16)
            nc.sync.dma_start(out=spb_sb, in_=spb[:, :, :].rearrange("p k m -> k p m"))

            # persistent psum: 4 banks for td pairs
            td_t = [
                tdp.tile([128, T], F32, tag="td", name=f"td{p}") for p in range(NP)
            ]
            acc_t = [
                accp.tile([128, T], BF16, tag="acc", name=f"acc{i}")
                for i in range(IT)
            ]

            # ---------- load x (hi/lo bf16) ----------
            x_sb = xp.tile([128, HC, T], BF16)
            nc.sync.dma_start(
                out=x_sb, in_=xh[:, :].rearrange("(c p) t -> p c t", p=128)
            )

            we_b = []  # [128,T] bf16 per expert
            wep_b = []  # [128,T] bf16 per pair (rows 0:64 = e even, 64:128 = e odd)
            tAgu = []  # (tAg, tAu) [64,T] bf16 per expert
            with tc.tile_pool(name="rt", bufs=1) as rt:
                xl_sb = rt.tile([128, HC, T], BF16, tag="xl")
                nc.sync.dma_start(
                    out=xl_sb, in_=xl[:, :].rearrange("(c p) t -> p c t", p=128)
                )
                cwh_sb = rt.tile([128, HC, E], BF16, tag="cwh")
                nc.sync.dma_start(
                    out=cwh_sb, in_=cwh[:, :].rearrange("(c p) e -> p c e", p=128)
                )
                cwl_sb = rt.tile([128, HC, E], BF16, tag="cwl")
                nc.sync.dma_start(
                    out=cwl_sb, in_=cwl[:, :].rearrange("(c p) e -> p c e", p=128)
                )

                # conf logits: xh@Wh + xl@Wh + xh@Wl  (fp32-accurate)
                p_cf = pw.tile([128, T], F32, tag="big", name="p_cf")
                terms = [(cwh_sb, x_sb), (cwh_sb, xl_sb), (cwl_sb, x_sb)]
                n = 0
                for w_sb, m_sb in terms:
                    for hc in range(HC):
                        nc.tensor.matmul(
                            p_cf[0:E, :],
                            w_sb[:, hc, :],
                            m_sb[:, hc, :],
                            start=(n == 0),
                            stop=(n == 3 * HC - 1),
                        )
                        n += 1
                conf = rt.tile([E, T], F32, tag="conf")
                nc.scalar.activation(conf, p_cf[0:E, :], AF.Sigmoid, bias=cb_sb)
                bids = rt.tile([E, T], F32, tag="bids")
                nc.vector.tensor_scalar(bids, conf, wl_sb, None, op0=OP.mult)

                # tA = x @ [gate_A | up_A] per expert -- emitted here so these
                # 128 dependency-free matmuls hide the routing chain's
                # ACT/DVE latency on the in-order PE.
                with tc.tile_pool(name="wga", bufs=2) as wga:
                    for e in range(E):
                        ga_sb = wga.tile([128, HC, 2 * R], BF16, tag="guA")
                        nc.sync.dma_start(
                            out=ga_sb,
                            in_=guA[e, :, :].rearrange("(c p) r -> p c r", p=128),
                        )
                        p_tA = pw.tile([128, T], F32, tag="big", name=f"ptA{e}")
                        for hc in range(HC):
                            nc.tensor.matmul(
                                p_tA,
                                ga_sb[:, hc, :],
                                x_sb[:, hc, :],
                                start=(hc == 0),
                                stop=(hc == HC - 1),
                            )
                        tAg_sb = tAp.tile([64, T], BF16, tag="tAg", name=f"tAg{e}")
                        nc.scalar.copy(tAg_sb, p_tA[0:64, :])
                        tAu_sb = tAp.tile([64, T], BF16, tag="tAu", name=f"tAu{e}")
                        nc.scalar.copy(tAu_sb, p_tA[64:128, :])
                        tAgu.append((tAg_sb, tAu_sb))

                # transpose bids to token space: [128 tok, 4 chunk, E] fp32
                TC = T // 128
                bidsT = rt.tile([128, TC, E], F32, tag="bidsT")
                for tc_i in range(TC):
                    p_bt = pw.tile([128, T], F32, tag="big", name=f"p_bt{tc_i}")
                    nc.tensor.transpose(
                        p_bt[:, 0:E], bids[:, tc_i * 128 : (tc_i + 1) * 128], id8_sb
                    )
                    nc.vector.tensor_copy(bidsT[:, tc_i, :], p_bt[:, 0:E])

                def fmax8(src, nametag):
                    """max over the expert free-dim: [128,TC,8] -> [128,TC,1]"""
                    m4 = rt.tile([128, TC, 4], F32, tag=nametag + "4", name=nametag + "4")
                    nc.vector.tensor_tensor(
                        m4, src[:, :, 0:4], src[:, :, 4:8], op=OP.max
                    )
                    m2_ = rt.tile([128, TC, 2], F32, tag=nametag + "2", name=nametag + "2")
                    nc.vector.tensor_tensor(
                        m2_, m4[:, :, 0:2], m4[:, :, 2:4], op=OP.max
                    )
                    m = rt.tile([128, TC, 1], F32, tag=nametag + "m", name=nametag + "m")
                    nc.vector.tensor_tensor(
                        m, m2_[:, :, 0:1], m2_[:, :, 1:2], op=OP.max
                    )
                    return m

                def bc(m):  # broadcast [128,TC,1] over expert free-dim
                    return m.broadcast_to([128, TC, E])

                m1 = fmax8(bidsT, "m1")
                mask1 = rt.tile([128, TC, E], F32, tag="mask1")
                nc.vector.tensor_tensor(mask1, bidsT, bc(m1), op=OP.is_equal)
                bids2 = rt.tile([128, TC, E], F32, tag="bids2")
                nc.vector.scalar_tensor_tensor(
                    bids2, mask1, -1e6, bidsT, op0=OP.mult, op1=OP.add
                )
                m2 = fmax8(bids2, "m2")
                mask2 = rt.tile([128, TC, E], F32, tag="mask2")
                nc.vector.tensor_tensor(mask2, bids2, bc(m2), op=OP.is_equal)

                d12 = rt.tile([128, TC, 1], F32, tag="d12")
                nc.vector.tensor_sub(d12, m1, m2)
                w1 = rt.tile([128, TC, 1], F32, tag="w1")
                nc.scalar.activation(w1, d12, AF.Sigmoid)
                w2 = rt.tile([128, TC, 1], F32, tag="w2")
                nc.scalar.activation(w2, d12, AF.Sigmoid, scale=-1.0)
                wea = rt.tile([128, TC, E], F32, tag="wea")
                nc.vector.tensor_mul(wea, mask1, bc(w1))
                web8 = rt.tile([128, TC, E], F32, tag="web8")
                nc.vector.tensor_mul(web8, mask2, bc(w2))
                we8T = rt.tile([128, TC, E], BF16, tag="we8T")
                nc.vector.tensor_add(we8T, wea, web8)

                # transpose back to [E, T] bf16
                we8 = rt.tile([E, T], BF16, tag="we8")
                for tc_i in range(TC):
                    p_wt = pw.tile([128, T], F32, tag="big", name=f"p_wt{tc_i}")
                    pv = p_wt.bitcast(BF16)[0:E, 0:128]
                    nc.tensor.transpose(pv, we8T[:, tc_i, :], id_sb)
                    nc.vector.tensor_copy(
                        we8[:, tc_i * 128 : (tc_i + 1) * 128], pv
                    )

                # broadcast each expert row to 128 partitions via K=8 select-matmul
                for e in range(E):
                    p_web = pw.tile([128, T], F32, tag="big", name=f"pweb{e}")
                    nc.tensor.matmul(p_web, seb_sb[:, e, :], we8, start=True, stop=True)
                    wt = webp.tile([128, T], BF16, tag="web", name=f"web{e}")
                    nc.scalar.copy(wt, p_web)
                    we_b.append(wt)
                for p in range(NP):
                    p_wep = pw.tile([128, T], F32, tag="big", name=f"pwep{p}")
                    nc.tensor.matmul(p_wep, spb_sb[:, p, :], we8, start=True, stop=True)
                    wt = webp.tile([128, T], BF16, tag="wep", name=f"wep{p}")
                    nc.scalar.copy(wt, p_wep)
                    wep_b.append(wt)

            # ---------- main loop over I chunks ----------
            acc_fn = nc.gpsimd if ACC_ENG == "gpsimd" else nc.vector
            hw_fn = nc.gpsimd if HW_ENG == "gpsimd" else nc.vector
            with (
                tc.tile_pool(name="wgw", bufs=2) as wgw,
                tc.tile_pool(name="wb", bufs=2) as wbp,
                tc.tile_pool(name="wdA", bufs=2) as wdAp,
                tc.tile_pool(name="bsb", bufs=2) as bsb,
                tc.tile_pool(name="ew", bufs=3) as ew,
            ):
                # td matmuls are emitted TDLAG experts behind their h
                # producers (crossing it boundaries) so the PE never waits
                # on the ACT->DVE chain that computes h. The hw/acc DVE ops
                # are likewise deferred WALAG experts so queued h ops (which
                # gate td matmuls) aren't stuck behind them on the in-order
                # DVE.
                td_pending = []  # list of emit closures, FIFO
                wa_pending = []  # deferred hw/acc emits, FIFO

                def make_td(it_, e_, dA_tile, h_tile):
                    def go():
                        p = e_ // 2
                        half = (e_ % 2) * 64
                        nc.tensor.matmul(
                            td_t[p][half : half + 64, :],
                            dA_tile[:, e_, :],
                            h_tile,
                            start=(it_ == 0),
                            stop=(it_ == IT - 1),
                            tile_position=(0, half),
                            skip_group_check=True,
                        )

                    return go

                def pop_td():
                    if td_pending:
                        td_pending.pop(0)()

                def make_wa(it_, e_, h_tile):
                    def go():
                        if e_ == 0:
                            hw_fn.tensor_mul(acc_t[it_], h_tile, we_b[e_])
                        else:
                            hw_t = ew.tile(
                                [128, T], BF16, tag="hw", bufs=WALAG + 2,
                                name="hw_t",
                            )
                            hw_fn.tensor_mul(hw_t, h_tile, we_b[e_])
                            acc_fn.tensor_add(acc_t[it_], acc_t[it_], hw_t)

                    return go

                def pop_wa():
                    if wa_pending:
                        wa_pending.pop(0)()

                for it in range(IT):
                    bg_w = wgw.tile([128, HC, 128], BF16, tag="bgw")
                    nc.sync.dma_start(
                        out=bg_w,
                        in_=bgate[:, it * 128 : (it + 1) * 128].rearrange(
                            "(c p) i -> p c i", p=128
                        ),
                    )
                    bu_w = wgw.tile([128, HC, 128], BF16, tag="buw")
                    nc.sync.dma_start(
                        out=bu_w,
                        in_=bup[:, it * 128 : (it + 1) * 128].rearrange(
                            "(c p) i -> p c i", p=128
                        ),
                    )
                    p_bg = pw.tile([128, T], F32, tag="big", name="p_bg")
                    p_bu = pw.tile([128, T], F32, tag="big", name="p_bu")
                    for hc in range(HC):
                        nc.tensor.matmul(
                            p_bg, bg_w[:, hc, :], x_sb[:, hc, :],
                            start=(hc == 0), stop=(hc == HC - 1),
                        )
                    pop_td()
                    for hc in range(HC):
                        nc.tensor.matmul(
                            p_bu, bu_w[:, hc, :], x_sb[:, hc, :],
                            start=(hc == 0), stop=(hc == HC - 1),
                        )
                    pop_td()
                    bg_s = bsb.tile([128, T], BF16, tag="bgs")
                    nc.scalar.copy(bg_s, p_bg)
                    bu_s = bsb.tile([128, T], BF16, tag="bus")
                    nc.scalar.copy(bu_s, p_bu)

                    guB_s = wbp.tile([64, E, 2, 128], BF16, tag="guB")
                    nc.sync.dma_start(
                        out=guB_s,
                        in_=guB[:, :, it * 128 : (it + 1) * 128].rearrange(
                            "e (gu r) i -> r e gu i", gu=2
                        ),
                    )
                    dA_s = wdAp.tile([128, E, R], BF16, tag="dA")
                    nc.sync.dma_start(
                        out=dA_s,
                        in_=dA[:, it * 128 : (it + 1) * 128, :].rearrange(
                            "e p r -> p e r"
                        ),
                    )

                    for e in range(E):
                        # g_e = base_g + lora_g in PSUM (identity-matmul trick,
                        # or ACT writes the base into the bank and the lora
                        # matmul accumulates on top)
                        p_g = pw.tile([128, T], F32, tag="big", name="p_g")
                        if IDG == "act":
                            nc.scalar.copy(p_g, bg_s)
                            nc.tensor.matmul(
                                p_g, guB_s[:, e, 0, :], tAgu[e][0],
                                start=False, stop=True, skip_group_check=True,
                            )
                        else:
                            nc.tensor.matmul(p_g, id_sb, bg_s, start=True, stop=False)
                            nc.tensor.matmul(
                                p_g, guB_s[:, e, 0, :], tAgu[e][0],
                                start=False, stop=True,
                            )
                        p_u = pw.tile([128, T], F32, tag="big", name="p_u")
                        if IDU:
                            nc.tensor.matmul(p_u, id_sb, bu_s, start=True, stop=False)
                            nc.tensor.matmul(
                                p_u, guB_s[:, e, 1, :], tAgu[e][1],
                                start=False, stop=True,
                            )
                        else:
                            nc.tensor.matmul(
                                p_u, guB_s[:, e, 1, :], tAgu[e][1],
                                start=True, stop=True,
                            )
                        sg = ew.tile([128, T], BF16, tag="sg")
                        nc.scalar.activation(sg, p_g, AF.Silu)
                        if IDU:
                            h_t = ew.tile([128, T], BF16, tag="h", bufs=HLAG)
                            nc.vector.tensor_mul(h_t, sg, p_u)
                        elif UPATH == "act":
                            # ACT drains PSUM so both DVE ops run in the
                            # fast all-SBUF bf16 mode
                            lu_s = ew.tile([128, T], BF16, tag="lu")
                            nc.scalar.copy(lu_s, p_u)
                            u_t = ew.tile([128, T], BF16, tag="u")
                            nc.vector.scalar_tensor_tensor(
                                u_t, lu_s, 1.0, bu_s, op0=OP.bypass, op1=OP.add
                            )
                            h_t = ew.tile([128, T], BF16, tag="h", bufs=HLAG)
                            nc.vector.tensor_mul(h_t, sg, u_t)
                        else:
                            u_t = ew.tile([128, T], BF16, tag="u")
                            nc.vector.scalar_tensor_tensor(
                                u_t, p_u, 1.0, bu_s, op0=OP.bypass, op1=OP.add
                            )
                            h_t = ew.tile([128, T], BF16, tag="h", bufs=HLAG)
                            nc.vector.tensor_mul(h_t, sg, u_t)
                        td_pending.append(make_td(it, e, dA_s, h_t))
                        if len(td_pending) > TDLAG:
                            pop_td()
                        wa_pending.append(make_wa(it, e, h_t))
                        if len(wa_pending) > WALAG:
                            pop_wa()
                while td_pending:
                    pop_td()
                while wa_pending:
                    pop_wa()

            # ---------- apply routing weights to td pairs ----------
            tdw = []
            with tc.tile_pool(name="tdw", bufs=NP) as tdwp:
                for p in range(NP):
                    t_w = tdwp.tile([128, T], BF16, tag="tdw", name=f"tdw{p}")
                    nc.vector.tensor_mul(t_w, td_t[p], wep_b[p])
                    tdw.append(t_w)

                # ---------- down projection ----------
                with (
                    tc.tile_pool(name="wd", bufs=2) as wd,
                    tc.tile_pool(name="wdB", bufs=2) as wdB,
                    tc.tile_pool(name="osb", bufs=3) as osb,
                ):
                    for hc in range(HC):
                        bd_s = wd.tile([128, IT, 128], BF16, tag="bd")
                        nc.sync.dma_start(
                            out=bd_s,
                            in_=bdown[
                                0 : IT * 128, hc * 128 : (hc + 1) * 128
                            ].rearrange("(c p) h -> p c h", p=128),
                        )
                        dB_s = wdB.tile([128, NP, 128], BF16, tag="dB")
                        nc.sync.dma_start(
                            out=dB_s,
                            in_=dBp[:, :, hc * 128 : (hc + 1) * 128].rearrange(
                                "p r h -> r p h"
                            ),
                        )
                        p_o = pw.tile([128, T], F32, tag="big", name="p_o")
                        for it in range(IT):
                            nc.tensor.matmul(
                                p_o, bd_s[:, it, :], acc_t[it],
                                start=(it == 0), stop=False,
                            )
                        for p in range(NP):
                            nc.tensor.matmul(
                                p_o, dB_s[:, p, :], tdw[p],
                                start=False, stop=(p == NP - 1),
                            )
                        o_s = osb.tile([128, T], F32, tag="o")
                        nc.scalar.copy(o_s, p_o)
                        nc.sync.dma_start(
                            out=outT[hc * 128 : (hc + 1) * 128, :], in_=o_s
                        )
    nc.compile()
    return nc


@functools.lru_cache(maxsize=1)
def _get_module():
    return build_module()


def _host_prep(inputs):
    f32 = np.float32
    x = np.ascontiguousarray(np.asarray(inputs["hidden_states"], f32)).reshape(
        N_TOK, H
    )
    x_hi = x.astype(BFNP)
    x_lo = (x - x_hi.astype(f32)).astype(BFNP)
    cw = np.asarray(inputs["conf_W"], f32).T  # [H, E]
    cw_hi = cw.astype(BFNP)
    cw_lo = (cw - cw_hi.astype(f32)).astype(BFNP)
    gate_A = np.asarray(inputs["gate_A"], f32)
    gate_B = np.asarray(inputs["gate_B"], f32)
    up_A = np.asarray(inputs["up_A"], f32)
    up_B = np.asarray(inputs["up_B"], f32)
    down_A = np.asarray(inputs["down_A"], f32)
    down_B = np.asarray(inputs["down_B"], f32) * f32(SCALING)

    seb = np.zeros((E, E, 128), BFNP)
    for e in range(E):
        seb[e, e, :] = 1.0
    spb = np.zeros((NP, E, 128), BFNP)
    for p in range(NP):
        spb[p, 2 * p, 0:64] = 1.0
        spb[p, 2 * p + 1, 64:128] = 1.0
    dBpair = np.stack(
        [np.concatenate([down_B[2 * p], down_B[2 * p + 1]], axis=0) for p in range(NP)]
    )  # [NP, 2R, H]

    shared = {
        "cwh": np.ascontiguousarray(cw_hi),
        "cwl": np.ascontiguousarray(cw_lo),
        "conf_b": np.ascontiguousarray(
            np.asarray(inputs["conf_b"], f32).reshape(E, 1)
        ),
        "wealth": np.ascontiguousarray(
            np.asarray(inputs["expert_wealth"], f32).reshape(E, 1)
        ),
        "id8": np.eye(E, dtype=f32),
        "seb": seb,
        "spb": spb,
        "guA": np.ascontiguousarray(
            np.concatenate([gate_A, up_A], axis=2).astype(BFNP)
        ),
        "guB": np.ascontiguousarray(
            (np.concatenate([gate_B, up_B], axis=1) * f32(SCALING)).astype(BFNP)
        ),
        "bgate": np.ascontiguousarray(np.asarray(inputs["base_gate"], f32).astype(BFNP)),
        "bup": np.ascontiguousarray(np.asarray(inputs["base_up"], f32).astype(BFNP)),
        "bdown": np.ascontiguousarray(
            np.asarray(inputs["base_down"], f32).astype(BFNP)
        ),
        "dA": np.ascontiguousarray(down_A.astype(BFNP)),
        "dBp": np.ascontiguousarray(dBpair.astype(BFNP)),
        "ident": np.eye(128, dtype=BFNP),
    }
    in_maps = []
    for c in range(N_CORES):
        m = dict(shared)
        m["xh"] = np.ascontiguousarray(x_hi[c * T : (c + 1) * T, :].T)
        m["xl"] = np.ascontiguousarray(x_lo[c * T : (c + 1) * T, :].T)
        in_maps.append(m)
    return in_maps


def kernel(**inputs) -> np.ndarray:
    nc = _get_module()
    in_maps = _host_prep(inputs)
    res = run_bass_kernel_spmd(nc, in_maps, core_ids=list(range(N_CORES)))
    parts = [np.asarray(r["outT"], np.float32).T for r in res.results]
    return np.concatenate(parts, axis=0).reshape(B, S, H)
